# revision 24
# baseline (speedup 1.0000x reference)
"""Trainium2 Bass kernel for the RNN-T style joint-network decoder.

Math (per batch element n):
    e  = enc @ W_enc.T                  # (Ti, J)
    d  = dec @ W_dec.T + b              # (To, J)
    A  = tanh(e[:,None,:] + d[None,:,:])# (Ti, To, J)
    out= A @ W_out.T                    # (Ti, To, V)

Sharding: data-parallel over N across the 8 cores (N == 8).

Per-core kernel layout strategy:
  - everything contracted over J/D lives with J/D on SBUF partitions
  - eT[j,t], dT[j,u] computed by matmuls from host-pretransposed inputs
  - A^T[j, col] (col = t*To+u) built in t-blocks: one broadcast DVE add +
    one large ACT tanh per j-chunk
  - final matmul: lhsT = A^T 128-column batches (stationary), rhs =
    W_out.T (moving, N=512), float32r for 1-cycle/row PE throughput,
    accumulate K=512 over 4 PSUM-accumulating matmuls
  - PSUM -> SBUF copies split across Vector/Scalar engines, then one
    contiguous 512KB DMA per 128-row batch
"""

import os
from contextlib import ExitStack

import numpy as np

import concourse.bass as bass
import concourse.tile as tile
from concourse import bacc
from concourse import mybir
from concourse.bass_utils import run_bass_kernel_spmd

P = 128

N_CORES = 8
TI = 200
TO = 100
J = 512
D = 512
V = 1024
TBLK = 32  # t's per A staging block; TBLK*TO must be divisible by P ideally

MM_DTYPE = os.environ.get("KERNEL_MM_DTYPE", "bf16")  # f32r | bf16 | f32


def build_nc(Ti=TI, To=TO, Jd=J, Dd=D, Vd=V, tblk=TBLK, mm_dtype=MM_DTYPE, timing=False, reps_const=0, ablate=""):
    f32 = mybir.dt.float32
    nj = Jd // P  # j-chunks
    nd = Dd // P  # d-chunks
    nc = bacc.Bacc(trn_type="TRN2", target_bir_lowering=False, debug=False)

    wout_dt = mybir.dt.bfloat16 if mm_dtype == "bf16" else f32
    in_dt = wout_dt  # stage-1 operands share the matmul dtype
    encT_d = nc.dram_tensor("encT", [Dd, Ti], in_dt, kind="ExternalInput").ap()
    decT_d = nc.dram_tensor("decT", [Dd, To], in_dt, kind="ExternalInput").ap()
    WencT_d = nc.dram_tensor("WencT", [Dd, Jd], in_dt, kind="ExternalInput").ap()
    WdecT_d = nc.dram_tensor("WdecT", [Dd, Jd], in_dt, kind="ExternalInput").ap()
    b_d = nc.dram_tensor("b", [Jd], f32, kind="ExternalInput").ap()
    WoutT_d = nc.dram_tensor("WoutT", [Jd, Vd], wout_dt, kind="ExternalInput").ap()
    if timing:
        # timing variant: big out stays on-device, tiny external output, and
        # the whole compute body repeats `reps` times (runtime value) so a
        # wall-clock slope over reps gives the per-iteration HW time.
        out_d = nc.dram_tensor("out", [Ti * To, Vd], f32).ap()
        done_d = nc.dram_tensor("done", [1, 1], f32, kind="ExternalOutput").ap()
    else:
        out_d = nc.dram_tensor("out", [Ti * To, Vd], f32, kind="ExternalOutput").ap()

    # t-blocks
    blocks = []
    t0 = 0
    while t0 < Ti:
        tb = min(tblk, Ti - t0)
        blocks.append((t0, tb))
        t0 += tb

    with tile.TileContext(nc) as tc, ExitStack() as ctx:
        consts = ctx.enter_context(tc.tile_pool(name="consts", bufs=1))
        apool = ctx.enter_context(tc.tile_pool(name="apool", bufs=2))
        opool = ctx.enter_context(tc.tile_pool(name="opool", bufs=12))
        psum = ctx.enter_context(tc.tile_pool(name="psum", bufs=4, space="PSUM"))
        psumb = ctx.enter_context(tc.tile_pool(name="psumb", bufs=4, space="PSUM"))

        # ---- input loads ----
        encT = []
        decT = []
        WencT = []
        WdecT = []
        b_sb = []
        for k in range(nd):
            t = consts.tile([P, Ti], in_dt, tag=f"encT{k}")
            nc.sync.dma_start(t[:], encT_d[bass.ts(k, P), :])
            encT.append(t)
            t = consts.tile([P, To], in_dt, tag=f"decT{k}")
            nc.sync.dma_start(t[:], decT_d[bass.ts(k, P), :])
            decT.append(t)
            t = consts.tile([P, Jd], in_dt, tag=f"WencT{k}")
            nc.sync.dma_start(t[:], WencT_d[bass.ts(k, P), :])
            WencT.append(t)
            t = consts.tile([P, Jd], in_dt, tag=f"WdecT{k}")
            nc.sync.dma_start(t[:], WdecT_d[bass.ts(k, P), :])
            WdecT.append(t)
        for j in range(nj):
            t = consts.tile([P, 1], f32, tag=f"b{j}")
            nc.sync.dma_start(t[:], b_d[bass.ts(j, P)][:, None])
            b_sb.append(t)
        WoutT = []
        for j in range(nj):
            t = consts.tile([P, Vd], wout_dt, tag=f"WoutT{j}")
            nc.sync.dma_start(t[:], WoutT_d[bass.ts(j, P), :])
            WoutT.append(t)

        # dtype view for the big matmul
        if mm_dtype == "f32r":
            a_dt = f32
            w_mm = [w[:].bitcast(mybir.dt.float32r) for w in WoutT]

            def a_mm(ap):
                return ap.bitcast(mybir.dt.float32r)
        elif mm_dtype == "bf16":
            a_dt = mybir.dt.bfloat16
            w_mm = [w[:] for w in WoutT]

            def a_mm(ap):
                return ap
        else:  # full fp32 (slow PE path, reference-accurate)
            a_dt = f32
            w_mm = [w[:] for w in WoutT]

            def a_mm(ap):
                return ap

        NV = Vd // 512  # moving-operand chunks of the vocab dim

        def body():
            # stage 1: eT[j] = (W_enc @ enc^T)[j-chunk], dT likewise + bias
            eT = []
            dT = []
            for j in range(nj):
                ps = psum.tile([P, 512], f32, tag="ps", name=f"psE{j}")
                for k in range(nd):
                    nc.tensor.matmul(
                        ps[:, :Ti],
                        WencT[k][:, bass.ts(j, P)],
                        encT[k][:],
                        start=(k == 0),
                        stop=(k == nd - 1),
                    )
                t = consts.tile([P, Ti], f32, tag=f"eT{j}", name=f"eT{j}")
                nc.vector.tensor_copy(t[:], ps[:, :Ti])
                eT.append(t)
            for j in range(nj):
                ps = psumb.tile([P, 512], f32, tag="ps", name=f"psD{j}")
                for k in range(nd):
                    nc.tensor.matmul(
                        ps[:, :To],
                        WdecT[k][:, bass.ts(j, P)],
                        decT[k][:],
                        start=(k == 0),
                        stop=(k == nd - 1),
                    )
                t = consts.tile([P, To], f32, tag=f"dT{j}", name=f"dT{j}")
                nc.vector.tensor_scalar_add(t[:], ps[:, :To], b_sb[j][:])
                dT.append(t)

            if mm_dtype == "bf16":
                # dummy weight loads: absorb the WoutT DMA sync waits on PE so
                # no main-loop Matmult needs two sync-wait commands (HW limit 1)
                for k in range(nj):
                    nc.tensor.ldweights(w_mm[k][:, 0:P])

            # main loop over t-blocks
            for t0, tb in blocks:
                cb = tb * To
                # build A^T[j, t*To+u] = tanh(eT[j,t] + dT[j,u])
                A = []
                for j in range(nj):
                    # DVE add writes the matmul dtype directly; tanh in place.
                    # Saves the separate fp32 staging buffer (funds a deeper
                    # output pool) at ~2e-3 extra rounding on the pre-tanh sum.
                    a = apool.tile([P, tblk * To], a_dt, tag=f"A{j}", name=f"A{j}_{t0}")[:, :cb]
                    av = a.rearrange("p (t u) -> p t u", u=To)
                    e_b = eT[j][:, t0 : t0 + tb][:, :, None].to_broadcast((P, tb, To))
                    d_b = dT[j][:, None, :].to_broadcast((P, tb, To))
                    nc.vector.tensor_tensor(av, e_b, d_b, mybir.AluOpType.add)
                    nc.scalar.activation(a, a, mybir.ActivationFunctionType.Tanh)
                    A.append(a)

                # matmul m-batches of up to 128 columns
                ob_pair = None
                c0 = 0
                while c0 < cb:
                    mw = min(P, cb - c0)
                    pss = [(psum if v % 2 == 0 else psumb).tile([P, 512], f32, tag="ps", name=f"ps{t0}_{c0}_{v}") for v in range(NV)]
                    for k in range(nj):
                        lhsT = a_mm(A[k][:, c0 : c0 + mw])
                        for v in range(NV):
                            nc.tensor.matmul(
                                pss[v][:mw, :],
                                lhsT,
                                w_mm[k][:, bass.ts(v, 512)],
                                start=(k == 0),
                                stop=(k == nj - 1),
                            )
                    if "nocopy" not in ablate:
                        # stage 2 consecutive m-batches into one tile so the
                        # output DMA is a single contiguous 1 MB transfer
                        half = (c0 // P) % 2
                        if half == 0 or ob_pair is None or mw < P:
                            ob_pair = opool.tile([P, 2 * Vd], f32, tag="ob", name=f"ob{t0}_{c0}")
                        ob = ob_pair[:, bass.ts(half, Vd)] if mw == P else ob_pair[:, :Vd]
                        for v in range(NV):
                            # split PSUM->SBUF copies across the two free engines
                            if v % 2 == 0:
                                nc.vector.tensor_copy(ob[:mw, bass.ts(v, 512)], pss[v][:mw, :])
                            else:
                                nc.scalar.copy(ob[:mw, bass.ts(v, 512)], pss[v][:mw, :])
                        if "nodma" not in ablate:
                            if mw < P:
                                r0 = t0 * To + c0
                                nc.sync.dma_start(out_d[r0 : r0 + mw, :], ob[:mw, :])
                            elif half == 1:
                                r0 = t0 * To + c0 - P
                                dst = out_d[r0 : r0 + 2 * P, :].rearrange("(i p) v -> p i v", p=P)
                                src3 = ob_pair[:, :].rearrange("p (i v) -> p i v", v=Vd)
                                nc.sync.dma_start(dst, src3)
                            elif cb - c0 - P < P:
                                # no full partner batch follows: flush singly
                                r0 = t0 * To + c0
                                nc.sync.dma_start(out_d[r0 : r0 + P, :], ob_pair[:, :Vd])
                    c0 += mw

        if timing:
            with tc.For_i(0, reps_const, 1):
                body()
            dn = consts.tile([1, 1], f32, tag="done", name="done_sb")
            nc.vector.memset(dn[:], 1.0)
            nc.sync.dma_start(done_d[:, :], dn[:])
        else:
            body()

    nc.compile()
    return nc


_cache = {}


def _get_nc():
    key = "main"
    if key not in _cache:
        _cache[key] = build_nc()
    return _cache[key]


def prepare_in_maps(enc_out, dec_out, W_enc, W_dec, b_dec, W_out):
    enc_out = np.asarray(enc_out, dtype=np.float32)
    dec_out = np.asarray(dec_out, dtype=np.float32)
    W_enc = np.asarray(W_enc, dtype=np.float32)
    W_dec = np.asarray(W_dec, dtype=np.float32)
    b_dec = np.asarray(b_dec, dtype=np.float32)
    W_out = np.asarray(W_out, dtype=np.float32)

    WencT = np.ascontiguousarray(W_enc.T)  # [D, J]
    WdecT = np.ascontiguousarray(W_dec.T)
    WoutT = np.ascontiguousarray(W_out.T)  # [J, V]
    encT = [np.ascontiguousarray(enc_out[n].T) for n in range(N_CORES)]
    decT = [np.ascontiguousarray(dec_out[n].T) for n in range(N_CORES)]
    if MM_DTYPE == "bf16":
        import ml_dtypes

        bf16 = ml_dtypes.bfloat16
        WoutT = WoutT.astype(bf16)
        WencT = WencT.astype(bf16)
        WdecT = WdecT.astype(bf16)
        encT = [x.astype(bf16) for x in encT]
        decT = [x.astype(bf16) for x in decT]

    in_maps = []
    for n in range(N_CORES):
        in_maps.append(
            {
                "encT": encT[n],
                "decT": decT[n],
                "WencT": WencT,
                "WdecT": WdecT,
                "b": b_dec,
                "WoutT": WoutT,
            }
        )
    return in_maps


def run_spmd(in_maps, **kw):
    return run_bass_kernel_spmd(_get_nc(), in_maps, list(range(N_CORES)), **kw)


def gather(results):
    out = np.stack([results[n]["out"].reshape(TI, TO, V) for n in range(N_CORES)])
    return out.astype(np.float32)


def kernel(enc_out, dec_out, W_enc, W_dec, b_dec, W_out):
    in_maps = prepare_in_maps(enc_out, dec_out, W_enc, W_dec, b_dec, W_out)
    return gather(run_spmd(in_maps).results)


# revision 25
# speedup vs baseline: 1.1108x; 1.1108x over previous
"""Trainium2 Bass kernel for the RNN-T style joint-network decoder.

Math (per batch element n):
    e  = enc @ W_enc.T                  # (Ti, J)
    d  = dec @ W_dec.T + b              # (To, J)
    A  = tanh(e[:,None,:] + d[None,:,:])# (Ti, To, J)
    out= A @ W_out.T                    # (Ti, To, V)

Sharding: data-parallel over N across the 8 cores (N == 8).

Per-core kernel layout strategy:
  - everything contracted over J/D lives with J/D on SBUF partitions
  - eT[j,t], dT[j,u] computed by matmuls from host-pretransposed inputs
  - A^T[j, col] (col = t*To+u) built in t-blocks: one broadcast DVE add +
    one large ACT tanh per j-chunk
  - final matmul: lhsT = A^T 128-column batches (stationary), rhs =
    W_out.T (moving, N=512), float32r for 1-cycle/row PE throughput,
    accumulate K=512 over 4 PSUM-accumulating matmuls
  - PSUM -> SBUF copies split across Vector/Scalar engines, then one
    contiguous 512KB DMA per 128-row batch
"""

import os
from contextlib import ExitStack

import numpy as np

import concourse.bass as bass
import concourse.tile as tile
from concourse import bacc
from concourse import mybir
from concourse.bass_utils import run_bass_kernel_spmd

P = 128

N_CORES = 8
TI = 200
TO = 100
J = 512
D = 512
V = 1024
TBLK = 32  # t's per A staging block; TBLK*TO must be divisible by P ideally

MM_DTYPE = os.environ.get("KERNEL_MM_DTYPE", "bf16")  # f32r | bf16 | f32


def build_nc(Ti=TI, To=TO, Jd=J, Dd=D, Vd=V, tblk=TBLK, mm_dtype=MM_DTYPE, timing=False, reps_const=0, ablate=""):
    f32 = mybir.dt.float32
    nj = Jd // P  # j-chunks
    nd = Dd // P  # d-chunks
    nc = bacc.Bacc(trn_type="TRN2", target_bir_lowering=False, debug=False)

    wout_dt = mybir.dt.bfloat16 if mm_dtype == "bf16" else f32
    in_dt = wout_dt  # stage-1 operands share the matmul dtype
    encT_d = nc.dram_tensor("encT", [Dd, Ti], in_dt, kind="ExternalInput").ap()
    decT_d = nc.dram_tensor("decT", [Dd, To], in_dt, kind="ExternalInput").ap()
    WencT_d = nc.dram_tensor("WencT", [Dd, Jd], in_dt, kind="ExternalInput").ap()
    WdecT_d = nc.dram_tensor("WdecT", [Dd, Jd], in_dt, kind="ExternalInput").ap()
    b_d = nc.dram_tensor("b", [Jd], f32, kind="ExternalInput").ap()
    WoutT_d = nc.dram_tensor("WoutT", [Jd, Vd], wout_dt, kind="ExternalInput").ap()
    if timing:
        # timing variant: big out stays on-device, tiny external output, and
        # the whole compute body repeats `reps` times (runtime value) so a
        # wall-clock slope over reps gives the per-iteration HW time.
        out_d = nc.dram_tensor("out", [Ti * To, Vd], f32).ap()
        done_d = nc.dram_tensor("done", [1, 1], f32, kind="ExternalOutput").ap()
    else:
        out_d = nc.dram_tensor("out", [Ti * To, Vd], f32, kind="ExternalOutput").ap()

    # t-blocks
    blocks = []
    t0 = 0
    while t0 < Ti:
        tb = min(tblk, Ti - t0)
        blocks.append((t0, tb))
        t0 += tb

    with tile.TileContext(nc) as tc, ExitStack() as ctx:
        consts = ctx.enter_context(tc.tile_pool(name="consts", bufs=1))
        apool = ctx.enter_context(tc.tile_pool(name="apool", bufs=2))
        opool = ctx.enter_context(tc.tile_pool(name="opool", bufs=8))
        psum = ctx.enter_context(tc.tile_pool(name="psum", bufs=4, space="PSUM"))
        psumb = ctx.enter_context(tc.tile_pool(name="psumb", bufs=4, space="PSUM"))

        # ---- input loads ----
        encT = []
        decT = []
        WencT = []
        WdecT = []
        b_sb = []
        for k in range(nd):
            t = consts.tile([P, Ti], in_dt, tag=f"encT{k}")
            nc.sync.dma_start(t[:], encT_d[bass.ts(k, P), :])
            encT.append(t)
            t = consts.tile([P, To], in_dt, tag=f"decT{k}")
            nc.sync.dma_start(t[:], decT_d[bass.ts(k, P), :])
            decT.append(t)
            t = consts.tile([P, Jd], in_dt, tag=f"WencT{k}")
            nc.sync.dma_start(t[:], WencT_d[bass.ts(k, P), :])
            WencT.append(t)
            t = consts.tile([P, Jd], in_dt, tag=f"WdecT{k}")
            nc.sync.dma_start(t[:], WdecT_d[bass.ts(k, P), :])
            WdecT.append(t)
        for j in range(nj):
            t = consts.tile([P, 1], f32, tag=f"b{j}")
            nc.sync.dma_start(t[:], b_d[bass.ts(j, P)][:, None])
            b_sb.append(t)
        WoutT = []
        for j in range(nj):
            t = consts.tile([P, Vd], wout_dt, tag=f"WoutT{j}")
            nc.sync.dma_start(t[:], WoutT_d[bass.ts(j, P), :])
            WoutT.append(t)

        # dtype view for the big matmul
        if mm_dtype == "f32r":
            a_dt = f32
            w_mm = [w[:].bitcast(mybir.dt.float32r) for w in WoutT]

            def a_mm(ap):
                return ap.bitcast(mybir.dt.float32r)
        elif mm_dtype == "bf16":
            a_dt = mybir.dt.bfloat16
            w_mm = [w[:] for w in WoutT]

            def a_mm(ap):
                return ap
        else:  # full fp32 (slow PE path, reference-accurate)
            a_dt = f32
            w_mm = [w[:] for w in WoutT]

            def a_mm(ap):
                return ap

        NV = Vd // 512  # moving-operand chunks of the vocab dim

        def body():
            # stage 1: eT[j] = (W_enc @ enc^T)[j-chunk], dT likewise + bias
            eT = []
            dT = []
            for j in range(nj):
                ps = psum.tile([P, 512], f32, tag="ps", name=f"psE{j}")
                for k in range(nd):
                    nc.tensor.matmul(
                        ps[:, :Ti],
                        WencT[k][:, bass.ts(j, P)],
                        encT[k][:],
                        start=(k == 0),
                        stop=(k == nd - 1),
                    )
                t = consts.tile([P, Ti], f32, tag=f"eT{j}", name=f"eT{j}")
                nc.vector.tensor_copy(t[:], ps[:, :Ti])
                eT.append(t)
            for j in range(nj):
                ps = psumb.tile([P, 512], f32, tag="ps", name=f"psD{j}")
                for k in range(nd):
                    nc.tensor.matmul(
                        ps[:, :To],
                        WdecT[k][:, bass.ts(j, P)],
                        decT[k][:],
                        start=(k == 0),
                        stop=(k == nd - 1),
                    )
                t = consts.tile([P, To], f32, tag=f"dT{j}", name=f"dT{j}")
                nc.vector.tensor_scalar_add(t[:], ps[:, :To], b_sb[j][:])
                dT.append(t)

            if mm_dtype == "bf16":
                # dummy weight loads: absorb the WoutT DMA sync waits on PE so
                # no main-loop Matmult needs two sync-wait commands (HW limit 1)
                for k in range(nj):
                    nc.tensor.ldweights(w_mm[k][:, 0:P])

            # main loop over t-blocks
            for t0, tb in blocks:
                cb = tb * To
                # build A^T[j, t*To+u] = tanh(eT[j,t] + dT[j,u])
                A = []
                for j in range(nj):
                    s = apool.tile([P, tblk * To], f32, tag=f"S{j}", name=f"S{j}_{t0}", bufs=1)[:, :cb]
                    sv = s.rearrange("p (t u) -> p t u", u=To)
                    e_b = eT[j][:, t0 : t0 + tb][:, :, None].to_broadcast((P, tb, To))
                    d_b = dT[j][:, None, :].to_broadcast((P, tb, To))
                    if "noact" not in ablate:
                        nc.vector.tensor_tensor(sv, e_b, d_b, mybir.AluOpType.add)
                    if a_dt == f32:
                        if "noact" not in ablate:
                            nc.scalar.activation(s, s, mybir.ActivationFunctionType.Tanh)
                        a = s
                    else:
                        a = apool.tile([P, tblk * To], a_dt, tag=f"A{j}", name=f"A{j}_{t0}")[:, :cb]
                        if "noact" not in ablate:
                            nc.scalar.activation(a, s, mybir.ActivationFunctionType.Tanh)
                    A.append(a)

                # matmul m-batches of up to 128 columns
                ob_pair = None
                c0 = 0
                while c0 < cb:
                    mw = min(P, cb - c0)
                    pss = [(psum if v % 2 == 0 else psumb).tile([P, 512], f32, tag="ps", name=f"ps{t0}_{c0}_{v}") for v in range(NV)]
                    for k in range(nj):
                        lhsT = a_mm(A[k][:, c0 : c0 + mw])
                        for v in range(NV):
                            nc.tensor.matmul(
                                pss[v][:mw, :],
                                lhsT,
                                w_mm[k][:, bass.ts(v, 512)],
                                start=(k == 0),
                                stop=(k == nj - 1),
                            )
                    if "nocopy" not in ablate:
                        # stage 2 consecutive m-batches into one tile so the
                        # output DMA is a single contiguous 1 MB transfer
                        half = (c0 // P) % 2
                        if half == 0 or ob_pair is None or mw < P:
                            ob_pair = opool.tile([P, 2 * Vd], f32, tag="ob", name=f"ob{t0}_{c0}")
                        ob = ob_pair[:, bass.ts(half, Vd)] if mw == P else ob_pair[:, :Vd]
                        for v in range(NV):
                            # split PSUM->SBUF copies across the two free engines
                            if v % 2 == 0:
                                nc.vector.tensor_copy(ob[:mw, bass.ts(v, 512)], pss[v][:mw, :])
                            else:
                                nc.scalar.copy(ob[:mw, bass.ts(v, 512)], pss[v][:mw, :])
                        if "nodma" not in ablate:
                            if mw < P:
                                r0 = t0 * To + c0
                                nc.sync.dma_start(out_d[r0 : r0 + mw, :], ob[:mw, :])
                            elif half == 1:
                                r0 = t0 * To + c0 - P
                                dst = out_d[r0 : r0 + 2 * P, :].rearrange("(i p) v -> p i v", p=P)
                                src3 = ob_pair[:, :].rearrange("p (i v) -> p i v", v=Vd)
                                nc.sync.dma_start(dst, src3)
                            elif cb - c0 - P < P:
                                # no full partner batch follows: flush singly
                                r0 = t0 * To + c0
                                nc.sync.dma_start(out_d[r0 : r0 + P, :], ob_pair[:, :Vd])
                    c0 += mw

        if timing:
            with tc.For_i(0, reps_const, 1):
                body()
            dn = consts.tile([1, 1], f32, tag="done", name="done_sb")
            nc.vector.memset(dn[:], 1.0)
            nc.sync.dma_start(done_d[:, :], dn[:])
        else:
            body()

    nc.compile()
    return nc


_cache = {}


def _get_nc():
    key = "main"
    if key not in _cache:
        _cache[key] = build_nc()
    return _cache[key]


def prepare_in_maps(enc_out, dec_out, W_enc, W_dec, b_dec, W_out):
    enc_out = np.asarray(enc_out, dtype=np.float32)
    dec_out = np.asarray(dec_out, dtype=np.float32)
    W_enc = np.asarray(W_enc, dtype=np.float32)
    W_dec = np.asarray(W_dec, dtype=np.float32)
    b_dec = np.asarray(b_dec, dtype=np.float32)
    W_out = np.asarray(W_out, dtype=np.float32)

    WencT = np.ascontiguousarray(W_enc.T)  # [D, J]
    WdecT = np.ascontiguousarray(W_dec.T)
    WoutT = np.ascontiguousarray(W_out.T)  # [J, V]
    encT = [np.ascontiguousarray(enc_out[n].T) for n in range(N_CORES)]
    decT = [np.ascontiguousarray(dec_out[n].T) for n in range(N_CORES)]
    if MM_DTYPE == "bf16":
        import ml_dtypes

        bf16 = ml_dtypes.bfloat16
        WoutT = WoutT.astype(bf16)
        WencT = WencT.astype(bf16)
        WdecT = WdecT.astype(bf16)
        encT = [x.astype(bf16) for x in encT]
        decT = [x.astype(bf16) for x in decT]

    in_maps = []
    for n in range(N_CORES):
        in_maps.append(
            {
                "encT": encT[n],
                "decT": decT[n],
                "WencT": WencT,
                "WdecT": WdecT,
                "b": b_dec,
                "WoutT": WoutT,
            }
        )
    return in_maps


def run_spmd(in_maps, **kw):
    return run_bass_kernel_spmd(_get_nc(), in_maps, list(range(N_CORES)), **kw)


def gather(results):
    out = np.stack([results[n]["out"].reshape(TI, TO, V) for n in range(N_CORES)])
    return out.astype(np.float32)


def kernel(enc_out, dec_out, W_enc, W_dec, b_dec, W_out):
    in_maps = prepare_in_maps(enc_out, dec_out, W_enc, W_dec, b_dec, W_out)
    return gather(run_spmd(in_maps).results)
